# revision 48
# baseline (speedup 1.0000x reference)
"""Multi-head attention Trainium2 kernel (8-core SPMD), flipped-PV design.

Sharding: core c -> batch b = c//4, head-group g = c%4 (4 heads each).
Each core computes partial_out[S, D] = attn(4 heads) @ Wo[rows of its heads].
Host sums the 4 partials per batch + bo.

Cost-model-driven layout (CoreSim charges a matmul by OUTPUT free size only;
LdWeights is free; ACT = free-size * 0.83ns; DMA transfers serialize on one
shared device):
  - scores per (head, t-chunk, s-half): psum [128 t, 1024 s], contraction
    DK=64 from pair-stacked QT/KT [128, S] tiles (no partition duplication).
  - exp on ACT (scale 1/8 folded) -> u [128, 1024] bf16. ACT is ~saturated
    during attention: everything else is kept off ACT.
  - PV FLIPPED: out[s-chunk 128, 65] += U-block^T @ V4b-slice, with the
    exp'd scores as the 128x128 stationary and V (64 cols + ones col) as a
    65-row moving operand -> 16640 PE rows/head instead of 32768.
  - PV accumulators live as 65-col slices packed 7/7/2 into three psum bank
    tiles; col 64 is the softmax denominator (ones column of V4b).
  - normalize = DVE reciprocal + per-partition tensor_scalar_mul -> bf16.
  - PE-transpose (identity) normalized [128 s, 64] blocks -> [64, 128] bf16
    psum -> DVE copy assembles OT2[pair] = [128 k, S] for the Wo stationary.
  - Wo: stationary OT2 blocks, moving Wo rows; drains alternate ACT/DVE.
  - DMA: one shared device; issue alternates SP/ACT (prefix) and SP/Pool
    (steady) to hide per-instruction issue overhead; x loaded in s-slices
    sized so first scores start ~13us in.
"""

import os
import sys

import numpy as np

sys.path.insert(0, "/opt/trn_rl_repo")

import ml_dtypes

BF16 = ml_dtypes.bfloat16

_CACHE = {}


def _build_nc(S, D, DK, NH, with_bias=True):
    import concourse.bass as bass
    import concourse.mybir as mybir
    import concourse.tile as tile
    from concourse import bacc

    bf = mybir.dt.bfloat16
    f32 = mybir.dt.float32
    P = 128
    NPAIR = NH // 2          # 2
    KT = D // P              # 8 contraction chunks for projections
    TT = S // P              # 16 t-chunks
    NV = NH * DK             # 256
    E = DK + 1               # 65: V cols + ones col
    NSC = S // P             # 16 s-chunks of 128
    LAG = 3                  # steps between exp(k) and its PV batch

    nc = bacc.Bacc("TRN2", target_bir_lowering=False, debug=False)

    xqT = nc.declare_dram_parameter("xqT", [D, S], bf, isOutput=False)
    xkT = nc.declare_dram_parameter("xkT", [D, S], bf, isOutput=False)
    xvT = nc.declare_dram_parameter("xvT", [D, S], bf, isOutput=False)
    wq2 = nc.declare_dram_parameter("wq2", [NPAIR, P, D], bf, isOutput=False)
    wk2 = nc.declare_dram_parameter("wk2", [NPAIR, P, D], bf, isOutput=False)
    wv4 = nc.declare_dram_parameter("wv4", [P, KT * NV], bf, isOutput=False)
    bq2 = nc.declare_dram_parameter("bq2", [NPAIR, 1, P], bf, isOutput=False)
    bk2 = nc.declare_dram_parameter("bk2", [NPAIR, 1, P], bf, isOutput=False)
    bv4 = nc.declare_dram_parameter("bv4", [1, NV], bf, isOutput=False)
    wo2 = nc.declare_dram_parameter("wo2", [NPAIR, P, D], bf, isOutput=False)
    ident = nc.declare_dram_parameter("ident", [P, P], bf, isOutput=False)
    out_d = nc.declare_dram_parameter("out", [S, D], bf, isOutput=True)

    EXP = mybir.ActivationFunctionType.Exp
    scale = 1.0 / float(np.sqrt(DK))

    with tile.TileContext(nc) as tc:
        with (
            tc.tile_pool(name="consts", bufs=1) as consts,
            tc.tile_pool(name="wp", bufs=1) as wp,
            tc.tile_pool(name="xt", bufs=1) as xt,
            tc.tile_pool(name="qk", bufs=1) as qkp,
            tc.tile_pool(name="ub", bufs=22) as ub,
            tc.tile_pool(name="vb", bufs=1) as vbp,
            tc.tile_pool(name="otp", bufs=1) as otp,
            tc.tile_pool(name="sm", bufs=4) as smp,
            tc.tile_pool(name="outp", bufs=3) as outp,
            tc.tile_pool(name="ps_sc", bufs=2, space="PSUM") as ps_sc,
            tc.tile_pool(name="ps_pv", bufs=1, space="PSUM") as ps_pv,
            tc.tile_pool(name="ps_sp", bufs=1, space="PSUM") as ps_sp,
        ):
            # ---------------- constants + weights + x DMA streams ----------
            id_sb = consts.tile([P, P], bf, tag="ident")
            ones_s = consts.tile([1, S], bf, tag="ones_s")
            if with_bias:
                nc.vector.memset(ones_s[:], 1.0)

            # DMA issue alternation: prefix SP/ACT, steady SP/Pool
            _pref = [nc.sync, nc.scalar]
            _stdy = [nc.sync, nc.gpsimd]
            _di = [0]

            def dma(engs, out, in_):
                engs[_di[0] % 2].dma_start(out=out, in_=in_)
                _di[0] += 1

            dma(_pref, id_sb[:], ident[:])
            wq_sb, wk_sb, wo_sb = [], [], []
            bq_sb, bk_sb = [], []
            for p in range(NPAIR):
                wq_sb.append(wp.tile([P, D], bf, tag=f"wq{p}", name=f"wq{p}"))
                wk_sb.append(wp.tile([P, D], bf, tag=f"wk{p}", name=f"wk{p}"))
                wo_sb.append(wp.tile([P, D], bf, tag=f"wo{p}", name=f"wo{p}"))
                bq_sb.append(wp.tile([1, P], bf, tag=f"bq{p}", name=f"bq{p}"))
                bk_sb.append(wp.tile([1, P], bf, tag=f"bk{p}", name=f"bk{p}"))
            wv_sb = wp.tile([P, KT * NV], bf, tag="wv")
            bv_sb = wp.tile([1, NV], bf, tag="bv")

            dma(_pref, wq_sb[0][:], wq2[0])
            dma(_pref, wk_sb[0][:], wk2[0])
            if with_bias:
                dma(_pref, bq_sb[0][:], bq2[0])
                dma(_pref, bk_sb[0][:], bk2[0])
                dma(_pref, bv_sb[:], bv4[:])

            xq_sb = [xt.tile([P, S], bf, tag=f"xq{k}", name=f"xq{k}") for k in range(KT)]
            xk_sb = [xt.tile([P, S], bf, tag=f"xk{k}", name=f"xk{k}") for k in range(KT)]
            xv_sb = [xt.tile([P, S], bf, tag=f"xv{k}", name=f"xv{k}") for k in range(KT)]

            def ldx(engs, xsb, xd, c0, c1):
                for k in range(KT):
                    dma(engs, xsb[k][:, c0:c1], xd[k * P : (k + 1) * P, c0:c1])

            # DMA transfers serialize on the shared device: order = just-in-time
            # need order of the consuming chains. First scores only need KT
            # t 0:256 and QT s 0:1024, so the prefix is three small slices.
            ldx(_pref, xk_sb, xkT, 0, 256)        # KT t 0:256 (k-e0)
            ldx(_pref, xq_sb, xqT, 0, 512)        # q-s0
            ldx(_pref, xq_sb, xqT, 512, 1024)     # q-s1
            # steady stream (SP/Pool)
            ldx(_stdy, xk_sb, xkT, 256, 512)      # k-s0b (scores t>=2)
            ldx(_stdy, xk_sb, xkT, 512, 1024)     # k-s1 (scores t>=4)
            ldx(_stdy, xk_sb, xkT, 1024, 1536)    # k-s2
            ldx(_stdy, xk_sb, xkT, 1536, 2048)    # k-s3
            ldx(_stdy, xq_sb, xqT, 1024, 2048)    # QT s-half 1 (scores step 16)
            dma(_stdy, wv_sb[:], wv4[:])
            ldx(_stdy, xv_sb, xvT, 0, 512)        # V-groups 0,1 (step 16)
            ldx(_stdy, xv_sb, xvT, 512, 1024)     # V-groups 2,3
            ldx(_stdy, xv_sb, xvT, 1024, 2048)    # V-groups 4-7
            dma(_stdy, wq_sb[1][:], wq2[1])
            dma(_stdy, wk_sb[1][:], wk2[1])
            if with_bias:
                dma(_stdy, bq_sb[1][:], bq2[1])
                dma(_stdy, bk_sb[1][:], bk2[1])
            dma(_stdy, wo_sb[0][:], wo2[0])
            dma(_stdy, wo_sb[1][:], wo2[1])

            # ---------------- persistent SBUF tensors ----------------------
            QT2 = [qkp.tile([P, S], bf, tag=f"qt{p}", name=f"qt{p}") for p in range(NPAIR)]
            KT2 = [qkp.tile([P, S], bf, tag=f"kt{p}", name=f"kt{p}") for p in range(NPAIR)]
            V4b = [vbp.tile([P, NH * E], bf, tag=f"v4b{t}", name=f"v4b{t}") for t in range(TT)]
            OT2 = [otp.tile([P, S], bf, tag=f"ot{p}", name=f"ot{p}") for p in range(NPAIR)]

            # ---------------- building blocks -------------------------------
            def proj_chain_mm(dst_w, x_sb, c0, c1, psum, k0, k1, b_sb=None):
                """Projection chain piece: psum[:, 0:c1-c0] += W_chunk^T x[:, c0:c1]."""
                sl = slice(c0, c1)
                for k in range(k0, k1):
                    nc.tensor.matmul(
                        psum[:, 0 : c1 - c0],
                        dst_w[:, k * P : (k + 1) * P],
                        x_sb[k][:, sl],
                        start=(k == 0),
                        stop=(k == KT - 1 and not with_bias),
                    )
                if k1 == KT and with_bias:
                    nc.tensor.matmul(
                        psum[:, 0 : c1 - c0], b_sb[0:1, :], ones_s[0:1, sl],
                        start=False, stop=True,
                    )

            def proj_copy(dst, c0, c1, psum, eng):
                if eng == "act":
                    nc.scalar.copy(dst[:, c0:c1], psum[:, 0 : c1 - c0])
                else:
                    nc.vector.tensor_copy(dst[:, c0:c1], psum[:, 0 : c1 - c0])

            def v_chain(t, psum, off, first, last):
                """V projection for one t-chunk into psum[:, off:off+NV].

                Both t-chunks of a group share one bank => one shared
                accumulation group (first chain starts, last chain stops).
                """
                tsl = slice(t * P, (t + 1) * P)
                for k in range(KT):
                    nc.tensor.matmul(
                        psum[:, off : off + NV],
                        xv_sb[k][:, tsl],
                        wv_sb[:, k * NV : (k + 1) * NV],
                        start=(k == 0 and first),
                        stop=(k == KT - 1 and last and not with_bias),
                    )
                if with_bias:
                    nc.tensor.matmul(
                        psum[:, off : off + NV],
                        ones_s[0:1, tsl],
                        bv_sb[0:1, :],
                        start=False, stop=last,
                    )

            def v_epilogue(t, psum, off):
                vt = V4b[t]
                nc.vector.tensor_copy(
                    vt.rearrange("p (h e) -> p h e", e=E)[:, :, 0:DK],
                    psum[:, off : off + NV].rearrange("p (h d) -> p h d", d=DK),
                )
                nc.vector.memset(
                    vt.rearrange("p (h e) -> p h e", e=E)[:, :, DK:E], 1.0
                )

            # PV accumulator layout: chunks 0..6 -> bank A, 7..13 -> B, 14..15 -> C.
            # Bank C (tag pvC) is allocated LAZILY on first touch: the same tag
            # slot hosts the filler projection chains emitted earlier in the
            # head, so C's slot must enter the tag FIFO after them.
            def acc_ap(banks, c, h):
                if c < 7:
                    return banks[0][:, c * E : c * E + E]
                if c < 14:
                    return banks[1][:, (c - 7) * E : (c - 7) * E + E]
                if banks[2] is None:
                    banks[2] = ps_pv.tile([P, 512], f32, tag="pvC", name=f"pvC_{h}")
                return banks[2][:, (c - 14) * E : (c - 14) * E + E]

            # ---------------- per-head attention ---------------------------
            pend_norm = []  # closures producing normalize/transpose of prev head

            def attn_head(h, fillers, pv_banks, early_tail=None, prefill=None):
                """fillers: dict step -> list of closures (emitted at that step).

                early_tail (head 3): list of (step, closure) consuming this
                head's own normalize/transpose closures + early Wo chains,
                emitted while the second s-half is still attending.
                """
                p, r = h // 2, h % 2
                rp = slice(64 * r, 64 * (r + 1))
                # Two V-gated FIFOs (one per s-half). Half-1 batches may
                # overtake gated half-0 ones: banks A (chunks 0-6) are
                # half-0-only and C (14,15) half-1-only, so only bank B's
                # stop (c13, the last half-1 batch) must wait for half-0
                # to finish (c7 accumulates into B).
                pend = ([], [])       # (t, half, u_tile) per half
                v_ready = [TT if h > 0 else -1]
                u_of = {}

                def emit_pv(t, half, ut):
                    # PSUM zero regions are whole banks: one accumulation
                    # group per bank, started by its first-touched chunk
                    # (start marks the bank pending-zero, so every chunk's
                    # t=0 write lands on fresh zeros) and stopped by the
                    # last-touched chunk.
                    for j8 in range(8):
                        c = half * 8 + j8
                        nc.tensor.matmul(
                            acc_ap(pv_banks, c, h),
                            ut[:, j8 * P : (j8 + 1) * P],
                            V4b[t][:, h * E : (h + 1) * E],
                            start=(t == 0 and c in (0, 7, 14)),
                            stop=(t == TT - 1 and c in (6, 13, 15)),
                        )

                first0_done = [False]  # bank B group start = first half-0 batch

                def drain(nmax):
                    n = 0
                    while n < nmax:
                        if pend[0] and pend[0][0][0] <= v_ready[0]:
                            emit_pv(*pend[0].pop(0))
                            first0_done[0] = True
                        elif (
                            pend[1]
                            and first0_done[0]
                            and pend[1][0][0] <= v_ready[0]
                            and not (pend[1][0][0] == TT - 1 and pend[0])
                        ):
                            emit_pv(*pend[1].pop(0))
                        else:
                            break
                        n += 1

                for k in range(32):
                    half, t = k // 16, k % 16
                    # PV batch first: it has no psum-slot dependency, so it
                    # fills the PE wait for exp(k-2) to free the scores slot.
                    kq = k - LAG
                    if kq >= 0:
                        pend[kq // 16].append((kq % 16, kq // 16, u_of.pop(kq)))
                    drain(3 if len(pend[0]) + len(pend[1]) > 4 else 2)
                    if prefill:
                        for fn in prefill.get(k, ()):
                            fn(v_ready)
                    # scores + exp
                    sc_t = ps_sc.tile([P, 1024], f32, tag="sc", name=f"sc{h}_{k}")
                    for j in range(2):
                        s0 = half * 1024 + j * 512
                        nc.tensor.matmul(
                            sc_t[:, j * 512 : (j + 1) * 512],
                            KT2[p][rp, t * P : (t + 1) * P],
                            QT2[p][rp, s0 : s0 + 512],
                            start=True,
                            stop=True,
                        )
                    ut = ub.tile([P, 1024], bf, tag="u", name=f"u{h}_{k}")
                    nc.scalar.activation(ut[:], sc_t[:], EXP, scale=scale)
                    u_of[k] = ut
                    # fillers
                    for fn in fillers.get(k, ()):
                        fn(v_ready)
                    if early_tail:
                        for es, fn in early_tail:
                            if es == k:
                                fn(v_ready)
                # flush remaining PV work
                for k in range(32 - LAG, 32):
                    pend[k // 16].append((k % 16, k // 16, u_of.pop(k)))
                drain(10**9)
                assert not pend[0] and not pend[1], (h, pend)

            # normalize + transpose, chunk-range granular (head 3 early-tails
            # only bank A's chunks 0..6 — reading an open-group bank is illegal)
            def norm_range(h, pv_banks, obfs, c0, c1):
                for c in range(c0, c1):
                    a = acc_ap(pv_banks, c, h)
                    rsb = smp.tile([P, 1], f32, tag="r", bufs=4, name=f"r{h}_{c}")
                    nc.vector.reciprocal(rsb[:], a[:, DK:E])
                    ob = smp.tile([P, DK], bf, tag="o", bufs=18, name=f"o{h}_{c}")
                    nc.vector.tensor_scalar_mul(ob[:], a[:, 0:DK], rsb[:, 0:1])
                    obfs[c] = ob

            def tr_range(h, obfs, c0, c1):
                p, r = h // 2, h % 2
                n = c1 - c0
                trt = ps_sp.tile([64, n * P], bf, tag="sp", name=f"tr{h}_{c0}")
                for i in range(n):
                    nc.tensor.transpose(
                        trt[:, i * P : (i + 1) * P], obfs.pop(c0 + i), id_sb[:]
                    )
                nc.vector.tensor_copy(
                    OT2[p][64 * r : 64 * r + 64, c0 * P : c1 * P], trt[0:64, :]
                )

            def make_norm_tr(h, pv_banks):
                obfs = {}

                def norm(g):
                    return lambda _vr: norm_range(h, pv_banks, obfs, 4 * g, 4 * g + 4)

                def tr(g):
                    return lambda _vr: tr_range(h, obfs, 4 * g, 4 * g + 4)

                return [norm(g) for g in range(4)] + [tr(g) for g in range(4)], obfs

            # Wo for one s-chunk m: contraction over both pairs, 2x 512-wide.
            # mode "dve": during head-3's second half (ACT is busy with exps):
            #   psum from the freed pvA bank + sp bank, drains on DVE only.
            # mode "tail": ps_sc rotation, drains split ACT/DVE in parallel.
            def wo_m(m, mode):
                msl = slice(m * P, (m + 1) * P)
                ot_b = outp.tile([P, D], bf, tag="outt", name=f"outt{m}")
                if mode == "dve":
                    wops = [
                        ps_pv.tile([P, 512], f32, tag="pvA", name=f"woA{m}"),
                        ps_sp.tile([P, 512], f32, tag="sp", name=f"woB{m}"),
                    ]
                elif mode == "tail2":
                    # second psum slot set for the tail so Wo(m) does not
                    # wait on drain(m-2) through the 2-slot sc rotation
                    wops = [
                        ps_pv.tile([P, 512], f32, tag="pvA", name=f"woA{m}"),
                        ps_pv.tile([P, 512], f32, tag="pvB", name=f"woB{m}"),
                    ]
                else:
                    w = ps_sc.tile([P, D], f32, tag="sc", name=f"wop{m}")
                    wops = [w[:, 0:512], w[:, 512:1024]]
                for dj in range(D // 512):
                    dsl = slice(dj * 512, (dj + 1) * 512)
                    for p in range(NPAIR):
                        nc.tensor.matmul(
                            wops[dj][:, 0:512] if mode == "dve" else wops[dj],
                            OT2[p][:, msl],
                            wo_sb[p][:, dsl],
                            start=(p == 0),
                            stop=(p == NPAIR - 1),
                        )
                for dj in range(D // 512):
                    dsl = slice(dj * 512, (dj + 1) * 512)
                    src = wops[dj] if mode == "tail" else wops[dj][:, 0:512]
                    if mode != "dve" and dj == 0:
                        nc.scalar.copy(ot_b[:, dsl], src)
                    else:
                        nc.vector.tensor_copy(ot_b[:, dsl], src)
                    if mode != "dve" and m >= 14:
                        # last tiles: per-half DMA right after each drain so
                        # the final transfer is small
                        dma(_stdy, out_d[msl, dsl], ot_b[:, dsl])
                if not (mode != "dve" and m >= 14):
                    dma(_stdy, out_d[msl, :], ot_b[:])

            # ---------------- filler schedules ------------------------------
            # A projection chain for cols c0:c1 is emitted as ~427ns pieces
            # (2 contraction chunks each): ACT is saturated during attention
            # and any PE burst between two scores delays every later exp
            # (a one-way ratchet), so fillers must stay below per-step slack.
            def mk_qk(p, which, c0, c1, pool, tag, eng="dve", npc=2):
                w = (wq_sb if which == "q" else wk_sb)[p]
                bsb = (bq_sb if which == "q" else bk_sb)[p]
                dst = (QT2 if which == "q" else KT2)[p]
                x = xq_sb if which == "q" else xk_sb
                st = {}
                pieces = []
                for k0 in range(0, KT, npc):
                    def piece(_vr, k0=k0):
                        if k0 == 0:
                            st["ps"] = pool.tile(
                                [P, 512], f32, tag=tag,
                                name=f"pj_{which}{p}_{c0}",
                            )
                        proj_chain_mm(w, x, c0, c1, st["ps"], k0, k0 + npc, bsb)
                        if k0 + npc == KT:
                            proj_copy(dst, c0, c1, st["ps"], eng)
                    pieces.append(piece)
                return pieces

            def v_group(g):
                """chain t=2g | chain t=2g+1 + epilogue (one step apart)."""
                st = {}

                def a(vr):
                    st["ps"] = ps_sp.tile([P, 512], f32, tag="sp", name=f"vps{g}")
                    v_chain(2 * g, st["ps"], 0, True, False)

                def b(vr):
                    v_chain(2 * g + 1, st["ps"], NV, False, True)
                    v_epilogue(2 * g, st["ps"], 0)
                    v_epilogue(2 * g + 1, st["ps"], NV)
                    vr[0] = 2 * g + 1

                return a, b

            # prefix: projections needed by head-0 step 0 (q s 0:1024, k t 0:256)
            for piece in mk_qk(0, "k", 0, 256, ps_sp, "sp", "act", npc=4):
                piece(None)
            for piece in mk_qk(0, "q", 0, 512, ps_sc, "sc", "act", npc=4):
                piece(None)
            for piece in mk_qk(0, "q", 512, 1024, ps_sc, "sc", "dve", npc=4):
                piece(None)

            def sched(f, pieces, steps):
                for fn, s in zip(pieces, steps):
                    f.setdefault(s, []).append(fn)

            fill0 = {}
            # dual psum slots (pvC + sp) let chains overlap; piece steps are
            # JIT vs the DMA stream; copies land >= 1 step before first use.
            prefill0 = {}
            sched(fill0, mk_qk(0, "k", 256, 512, ps_pv, "pvC", npc=4), (0, 1))
            sched(fill0, mk_qk(0, "k", 512, 1024, ps_sp, "sp"), (2, 2, 3, 3))
            sched(fill0, mk_qk(0, "k", 1024, 1536, ps_pv, "pvC"), (6, 6, 7, 7))
            sched(fill0, mk_qk(0, "k", 1536, 2048, ps_sp, "sp"), (10, 10, 11, 11))
            sched(fill0, mk_qk(0, "q", 1024, 1536, ps_pv, "pvC"), (13, 13, 14, 14))
            # q-s3 is DMA-gated until ~step 15; its last piece + copy run in
            # the pre-scores slot of step 16 (program order = dep order).
            qs3 = mk_qk(0, "q", 1536, 2048, ps_sp, "sp")
            sched(fill0, qs3[:3], (14, 15, 15))
            sched(prefill0, qs3[3:], (16,))
            for g, s in enumerate((18, 20, 22, 24, 26, 28, 29, 30)):
                a, b = v_group(g)
                fill0.setdefault(s, []).append(a)
                fill0.setdefault(min(s + 1, 31), []).append(b)

            def head_fill(prev_norm_tr, qk_items, stride=5):
                f = {}
                for i, fn in enumerate(prev_norm_tr):
                    f.setdefault(i // 2, []).append(fn)
                step = 4
                for p, which, sc, tag in qk_items:
                    pool = ps_pv if tag == "pvC" else ps_sp
                    pieces = mk_qk(p, which, sc * 512, (sc + 1) * 512, pool, tag)
                    sched(f, pieces, range(step, step + 4))
                    step += stride
                return f

            # ---------------- run the four heads ----------------------------
            def pv_alloc(h):
                return [
                    ps_pv.tile([P, 512], f32, tag="pvA", name=f"pvA_{h}"),
                    ps_pv.tile([P, 512], f32, tag="pvB", name=f"pvB_{h}"),
                    None,  # pvC: lazy, see acc_ap
                ]

            banks0 = pv_alloc(0)
            attn_head(0, fill0, banks0, prefill=prefill0)
            nt0, _ = make_norm_tr(0, banks0)
            banks1 = pv_alloc(1)
            attn_head(
                1,
                head_fill(
                    nt0,
                    [(1, "q", 0, "pvC"), (1, "k", 0, "sp"), (1, "q", 1, "pvC"),
                     (1, "k", 1, "sp"), (1, "k", 2, "sp")],
                ),
                banks1,
            )
            nt1, _ = make_norm_tr(1, banks1)
            banks2 = pv_alloc(2)
            attn_head(
                2,
                head_fill(
                    nt1,
                    [(1, "k", 3, "pvC"), (1, "q", 2, "sp"), (1, "q", 3, "pvC")],
                    stride=4,
                ),
                banks2,
            )
            nt2, _ = make_norm_tr(2, banks2)
            banks3 = pv_alloc(3)
            obfs3 = {}
            # head 3: bank A's chunks 0..6 finish (group closed) once half-0
            # PV flushes (~step 18): normalize/transpose them and run Wo
            # m=0..6 during half 1. Chunk 7 shares bank B with half-1 chunks
            # (group still open) so it waits for the tail.
            early = [
                (19, lambda _vr: norm_range(3, banks3, obfs3, 0, 4)),
                (20, lambda _vr: norm_range(3, banks3, obfs3, 4, 7)),
                (20, lambda _vr: tr_range(3, obfs3, 0, 4)),
                (21, lambda _vr: tr_range(3, obfs3, 4, 7)),
            ]
            for m in range(7):
                early.append((22 + m, lambda _vr, m=m: wo_m(m, "dve")))
            attn_head(3, head_fill(nt2, []), banks3, early_tail=early)

            # ---------------- tail: chunks 7..15 of head 3, Wo m=7..15 ------
            # all normalizes/transposes first, then Wo streams through four
            # psum slots (sc x2 + pvA/pvB) without drain-WAR stalls
            norm_range(3, banks3, obfs3, 7, 8)
            tr_range(3, obfs3, 7, 8)
            wo_m(7, "tail")
            norm_range(3, banks3, obfs3, 8, 12)
            tr_range(3, obfs3, 8, 12)
            wo_m(8, "tail")
            norm_range(3, banks3, obfs3, 12, 16)
            wo_m(9, "tail2")
            tr_range(3, obfs3, 12, 16)
            for m in range(10, 16):
                wo_m(m, "tail" if m % 2 == 0 else "tail2")

    nc.finalize()
    return nc


def _prep_core_inputs(query, key, value, Wq, bq, Wk, bk, Wv, bv, Wo, b, g, NH, DK):
    """Host-side shard prep for core (b, g): transpose+cast, pack weights."""
    D = query.shape[2]
    h0 = g * NH
    sl = slice(h0, h0 + NH)
    Wq_g, Wk_g, Wv_g = Wq[sl], Wk[sl], Wv[sl]
    bq_g, bk_g, bv_g = bq[sl], bk[sl], bv[sl]
    NPAIR = NH // 2
    P = 128
    KT = D // P

    def pack_pair(W, bias):
        # [NPAIR, 128, D]: pair p cols = heads (2p, 2p+1) concat; k-major free
        w = np.concatenate(
            [
                np.concatenate([W[2 * p], W[2 * p + 1]], axis=1)[None]
                for p in range(NPAIR)
            ],
            axis=0,
        )  # [NPAIR, D, 128]
        w = w.reshape(NPAIR, KT, P, P).transpose(0, 2, 1, 3).reshape(NPAIR, P, D)
        bb = np.concatenate(
            [
                np.concatenate([bias[2 * p], bias[2 * p + 1]])[None, None]
                for p in range(NPAIR)
            ],
            axis=0,
        )  # [NPAIR, 1, 128]
        return w.astype(BF16), bb.astype(BF16)

    wq2, bq2 = pack_pair(Wq_g, bq_g)
    wk2, bk2 = pack_pair(Wk_g, bk_g)
    wv = np.concatenate([Wv_g[i] for i in range(NH)], axis=1)  # [D, NH*DK]
    NV = NH * DK
    wv4 = wv.reshape(KT, P, NV).transpose(1, 0, 2).reshape(P, KT * NV).astype(BF16)
    bv4 = np.concatenate([bv_g[i] for i in range(NH)])[None].astype(BF16)
    wo2 = (
        Wo[h0 * DK : (h0 + NH) * DK]
        .reshape(NPAIR, P, D)
        .astype(BF16)
    )
    return {
        "xqT": np.ascontiguousarray(query[b].T).astype(BF16),
        "xkT": np.ascontiguousarray(key[b].T).astype(BF16),
        "xvT": np.ascontiguousarray(value[b].T).astype(BF16),
        "wq2": wq2,
        "wk2": wk2,
        "wv4": wv4,
        "bq2": bq2,
        "bk2": bk2,
        "bv4": bv4,
        "wo2": wo2,
        "ident": np.eye(P, dtype=np.float32).astype(BF16),
    }


def kernel(query, key, value, Wq, bq, Wk, bk, Wv, bv, Wo, bo, _trace=False):
    from concourse.bass_utils import run_bass_kernel_spmd

    query = np.asarray(query, np.float32)
    key = np.asarray(key, np.float32)
    value = np.asarray(value, np.float32)
    B, S, D = query.shape
    H, _, DK = np.asarray(Wq).shape
    NCORE = 8
    GROUPS = NCORE // B
    NH = H // GROUPS

    with_bias = bool(
        np.any(np.asarray(bq)) or np.any(np.asarray(bk)) or np.any(np.asarray(bv))
    )
    ck = ("nc", with_bias)
    if ck not in _CACHE:
        _CACHE[ck] = _build_nc(S, D, DK, NH, with_bias=with_bias)
    nc = _CACHE[ck]

    in_maps = []
    for c in range(NCORE):
        b, g = c // GROUPS, c % GROUPS
        in_maps.append(
            _prep_core_inputs(
                np.asarray(query), np.asarray(key), np.asarray(value),
                np.asarray(Wq), np.asarray(bq), np.asarray(Wk), np.asarray(bk),
                np.asarray(Wv), np.asarray(bv), np.asarray(Wo), b, g, NH, DK,
            )
        )

    res = run_bass_kernel_spmd(nc, in_maps, list(range(NCORE)), trace=_trace)
    out = np.zeros((B, S, D), np.float32)
    for c in range(NCORE):
        out[c // GROUPS] += np.asarray(res.results[c]["out"], np.float32)
    out += np.asarray(bo, np.float32)[None, None, :]
    if _trace:
        _CACHE["last_results"] = res
    return out


# revision 54
# speedup vs baseline: 1.0072x; 1.0072x over previous
"""Multi-head attention Trainium2 kernel (8-core SPMD), flipped-PV design.

Sharding: core c -> batch b = c//4, head-group g = c%4 (4 heads each).
Each core computes partial_out[S, D] = attn(4 heads) @ Wo[rows of its heads].
Host sums the 4 partials per batch + bo.

Cost-model-driven layout (CoreSim charges a matmul by OUTPUT free size only;
LdWeights is free; ACT = free-size * 0.83ns; DMA transfers serialize on one
shared device):
  - scores per (head, t-chunk, s-half): psum [128 t, 1024 s], contraction
    DK=64 from pair-stacked QT/KT [128, S] tiles (no partition duplication).
  - exp on ACT (scale 1/8 folded) -> u [128, 1024] bf16. ACT is ~saturated
    during attention: everything else is kept off ACT.
  - PV FLIPPED: out[s-chunk 128, 65] += U-block^T @ V4b-slice, with the
    exp'd scores as the 128x128 stationary and V (64 cols + ones col) as a
    65-row moving operand -> 16640 PE rows/head instead of 32768.
  - PV accumulators live as 65-col slices packed 7/7/2 into three psum bank
    tiles; col 64 is the softmax denominator (ones column of V4b).
  - normalize = DVE reciprocal + per-partition tensor_scalar_mul -> bf16.
  - PE-transpose (identity) normalized [128 s, 64] blocks -> [64, 128] bf16
    psum -> DVE copy assembles OT2[pair] = [128 k, S] for the Wo stationary.
  - Wo: stationary OT2 blocks, moving Wo rows; drains alternate ACT/DVE.
  - DMA: one shared device; issue alternates SP/ACT (prefix) and SP/Pool
    (steady) to hide per-instruction issue overhead; x loaded in JIT-ordered
    s-slices so the first exp fires ~10.5us in.
"""

import os
import sys

import numpy as np

sys.path.insert(0, "/opt/trn_rl_repo")

import ml_dtypes

BF16 = ml_dtypes.bfloat16

_CACHE = {}


def _build_nc(S, D, DK, NH, with_bias=True):
    import concourse.bass as bass
    import concourse.mybir as mybir
    import concourse.tile as tile
    from concourse import bacc

    bf = mybir.dt.bfloat16
    f32 = mybir.dt.float32
    P = 128
    NPAIR = NH // 2          # 2
    KT = D // P              # 8 contraction chunks for projections
    TT = S // P              # 16 t-chunks
    NV = NH * DK             # 256
    E = DK + 1               # 65: V cols + ones col
    NSC = S // P             # 16 s-chunks of 128
    LAG = 3                  # steps between exp(k) and its PV batch

    nc = bacc.Bacc("TRN2", target_bir_lowering=False, debug=False)

    xqT = nc.declare_dram_parameter("xqT", [D, S], bf, isOutput=False)
    xkT = nc.declare_dram_parameter("xkT", [D, S], bf, isOutput=False)
    xvT = nc.declare_dram_parameter("xvT", [D, S], bf, isOutput=False)
    wq2 = nc.declare_dram_parameter("wq2", [NPAIR, P, D], bf, isOutput=False)
    wk2 = nc.declare_dram_parameter("wk2", [NPAIR, P, D], bf, isOutput=False)
    wv4 = nc.declare_dram_parameter("wv4", [P, KT * NV], bf, isOutput=False)
    bq2 = nc.declare_dram_parameter("bq2", [NPAIR, 1, P], bf, isOutput=False)
    bk2 = nc.declare_dram_parameter("bk2", [NPAIR, 1, P], bf, isOutput=False)
    bv4 = nc.declare_dram_parameter("bv4", [1, NV], bf, isOutput=False)
    wo2 = nc.declare_dram_parameter("wo2", [NPAIR, P, D], bf, isOutput=False)
    ident = nc.declare_dram_parameter("ident", [P, P], bf, isOutput=False)
    out_d = nc.declare_dram_parameter("out", [S, D], bf, isOutput=True)

    EXP = mybir.ActivationFunctionType.Exp
    scale = 1.0 / float(np.sqrt(DK))

    with tile.TileContext(nc) as tc:
        with (
            tc.tile_pool(name="consts", bufs=1) as consts,
            tc.tile_pool(name="wp", bufs=1) as wp,
            tc.tile_pool(name="xt", bufs=1) as xt,
            tc.tile_pool(name="qk", bufs=1) as qkp,
            tc.tile_pool(name="ub", bufs=22) as ub,
            tc.tile_pool(name="vb", bufs=1) as vbp,
            tc.tile_pool(name="otp", bufs=1) as otp,
            tc.tile_pool(name="sm", bufs=4) as smp,
            tc.tile_pool(name="outp", bufs=3) as outp,
            tc.tile_pool(name="ps_sc", bufs=2, space="PSUM") as ps_sc,
            tc.tile_pool(name="ps_pv", bufs=1, space="PSUM") as ps_pv,
            tc.tile_pool(name="ps_sp", bufs=1, space="PSUM") as ps_sp,
        ):
            # ---------------- constants + weights + x DMA streams ----------
            id_sb = consts.tile([P, P], bf, tag="ident")
            ones_s = consts.tile([1, S], bf, tag="ones_s")
            if with_bias:
                nc.vector.memset(ones_s[:], 1.0)

            # DMA issue alternation: prefix SP/ACT, steady SP/Pool
            _pref = [nc.sync, nc.scalar]
            _stdy = [nc.sync, nc.gpsimd]
            _di = [0]

            def dma(engs, out, in_):
                engs[_di[0] % 2].dma_start(out=out, in_=in_)
                _di[0] += 1

            dma(_pref, id_sb[:], ident[:])
            wq_sb, wk_sb, wo_sb = [], [], []
            bq_sb, bk_sb = [], []
            for p in range(NPAIR):
                wq_sb.append(wp.tile([P, D], bf, tag=f"wq{p}", name=f"wq{p}"))
                wk_sb.append(wp.tile([P, D], bf, tag=f"wk{p}", name=f"wk{p}"))
                wo_sb.append(wp.tile([P, D], bf, tag=f"wo{p}", name=f"wo{p}"))
                bq_sb.append(wp.tile([1, P], bf, tag=f"bq{p}", name=f"bq{p}"))
                bk_sb.append(wp.tile([1, P], bf, tag=f"bk{p}", name=f"bk{p}"))
            wv_sb = wp.tile([P, KT * NV], bf, tag="wv")
            bv_sb = wp.tile([1, NV], bf, tag="bv")

            dma(_pref, wq_sb[0][:], wq2[0])
            dma(_pref, wk_sb[0][:], wk2[0])
            if with_bias:
                dma(_pref, bq_sb[0][:], bq2[0])
                dma(_pref, bk_sb[0][:], bk2[0])
                dma(_pref, bv_sb[:], bv4[:])

            xq_sb = [xt.tile([P, S], bf, tag=f"xq{k}", name=f"xq{k}") for k in range(KT)]
            xk_sb = [xt.tile([P, S], bf, tag=f"xk{k}", name=f"xk{k}") for k in range(KT)]
            xv_sb = [xt.tile([P, S], bf, tag=f"xv{k}", name=f"xv{k}") for k in range(KT)]

            def ldx(engs, xsb, xd, c0, c1):
                for k in range(KT):
                    dma(engs, xsb[k][:, c0:c1], xd[k * P : (k + 1) * P, c0:c1])

            # DMA transfers serialize on the shared device: order = just-in-time
            # need order of the consuming chains. First scores only need KT
            # t 0:256 and QT s 0:1024, so the prefix is three small slices.
            ldx(_pref, xk_sb, xkT, 0, 256)        # KT t 0:256 (k-e0)
            ldx(_pref, xq_sb, xqT, 0, 512)        # q-s0
            ldx(_pref, xq_sb, xqT, 512, 1024)     # q-s1
            # steady stream (SP/Pool)
            ldx(_stdy, xk_sb, xkT, 256, 512)      # k-s0b (scores t>=2)
            ldx(_stdy, xk_sb, xkT, 512, 1024)     # k-s1 (scores t>=4)
            ldx(_stdy, xk_sb, xkT, 1024, 1536)    # k-s2
            ldx(_stdy, xk_sb, xkT, 1536, 2048)    # k-s3
            ldx(_stdy, xq_sb, xqT, 1024, 2048)    # QT s-half 1 (scores step 16)
            dma(_stdy, wv_sb[:], wv4[:])
            ldx(_stdy, xv_sb, xvT, 0, 512)        # V-groups 0,1 (step 16)
            ldx(_stdy, xv_sb, xvT, 512, 1024)     # V-groups 2,3
            ldx(_stdy, xv_sb, xvT, 1024, 2048)    # V-groups 4-7
            dma(_stdy, wq_sb[1][:], wq2[1])
            dma(_stdy, wk_sb[1][:], wk2[1])
            if with_bias:
                dma(_stdy, bq_sb[1][:], bq2[1])
                dma(_stdy, bk_sb[1][:], bk2[1])
            dma(_stdy, wo_sb[0][:], wo2[0])
            dma(_stdy, wo_sb[1][:], wo2[1])

            # ---------------- persistent SBUF tensors ----------------------
            QT2 = [qkp.tile([P, S], bf, tag=f"qt{p}", name=f"qt{p}") for p in range(NPAIR)]
            KT2 = [qkp.tile([P, S], bf, tag=f"kt{p}", name=f"kt{p}") for p in range(NPAIR)]
            V4b = [vbp.tile([P, NH * E], bf, tag=f"v4b{t}", name=f"v4b{t}") for t in range(TT)]
            OT2 = [otp.tile([P, S], bf, tag=f"ot{p}", name=f"ot{p}") for p in range(NPAIR)]

            # ---------------- building blocks -------------------------------
            def proj_chain_mm(dst_w, x_sb, c0, c1, psum, k0, k1, b_sb=None):
                """Projection chain piece: psum[:, 0:c1-c0] += W_chunk^T x[:, c0:c1]."""
                sl = slice(c0, c1)
                for k in range(k0, k1):
                    nc.tensor.matmul(
                        psum[:, 0 : c1 - c0],
                        dst_w[:, k * P : (k + 1) * P],
                        x_sb[k][:, sl],
                        start=(k == 0),
                        stop=(k == KT - 1 and not with_bias),
                    )
                if k1 == KT and with_bias:
                    nc.tensor.matmul(
                        psum[:, 0 : c1 - c0], b_sb[0:1, :], ones_s[0:1, sl],
                        start=False, stop=True,
                    )

            def proj_copy(dst, c0, c1, psum, eng):
                if eng == "act":
                    nc.scalar.copy(dst[:, c0:c1], psum[:, 0 : c1 - c0])
                else:
                    nc.vector.tensor_copy(dst[:, c0:c1], psum[:, 0 : c1 - c0])

            def v_chain(t, psum, off, first, last):
                """V projection for one t-chunk into psum[:, off:off+NV].

                Both t-chunks of a group share one bank => one shared
                accumulation group (first chain starts, last chain stops).
                """
                tsl = slice(t * P, (t + 1) * P)
                for k in range(KT):
                    nc.tensor.matmul(
                        psum[:, off : off + NV],
                        xv_sb[k][:, tsl],
                        wv_sb[:, k * NV : (k + 1) * NV],
                        start=(k == 0 and first),
                        stop=(k == KT - 1 and last and not with_bias),
                    )
                if with_bias:
                    nc.tensor.matmul(
                        psum[:, off : off + NV],
                        ones_s[0:1, tsl],
                        bv_sb[0:1, :],
                        start=False, stop=last,
                    )

            def v_epilogue(t, psum, off):
                vt = V4b[t]
                nc.vector.tensor_copy(
                    vt.rearrange("p (h e) -> p h e", e=E)[:, :, 0:DK],
                    psum[:, off : off + NV].rearrange("p (h d) -> p h d", d=DK),
                )
                nc.vector.memset(
                    vt.rearrange("p (h e) -> p h e", e=E)[:, :, DK:E], 1.0
                )

            # PV accumulator layout: chunks 0..6 -> bank A, 7..13 -> B, 14..15 -> C.
            # Bank C (tag pvC) is allocated LAZILY on first touch: the same tag
            # slot hosts the filler projection chains emitted earlier in the
            # head, so C's slot must enter the tag FIFO after them.
            def acc_ap(banks, c, h):
                if c < 7:
                    return banks[0][:, c * E : c * E + E]
                if c < 14:
                    return banks[1][:, (c - 7) * E : (c - 7) * E + E]
                if banks[2] is None:
                    banks[2] = ps_pv.tile([P, 512], f32, tag="pvC", name=f"pvC_{h}")
                return banks[2][:, (c - 14) * E : (c - 14) * E + E]

            # ---------------- per-head attention ---------------------------
            pend_norm = []  # closures producing normalize/transpose of prev head

            def attn_head(h, fillers, pv_banks, early_tail=None, prefill=None):
                """fillers: dict step -> list of closures (emitted at that step).

                early_tail (head 3): list of (step, closure) consuming this
                head's own normalize/transpose closures + early Wo chains,
                emitted while the second s-half is still attending.
                """
                p, r = h // 2, h % 2
                rp = slice(64 * r, 64 * (r + 1))
                # Two V-gated FIFOs (one per s-half). Half-1 batches may
                # overtake gated half-0 ones: banks A (chunks 0-6) are
                # half-0-only and C (14,15) half-1-only, so only bank B's
                # stop (c13, the last half-1 batch) must wait for half-0
                # to finish (c7 accumulates into B).
                pend = ([], [])       # (t, half, u_tile) per half
                v_ready = [TT if h > 0 else -1]
                u_of = {}

                def emit_pv(t, half, ut):
                    # PSUM zero regions are whole banks: one accumulation
                    # group per bank, started by its first-touched chunk
                    # (start marks the bank pending-zero, so every chunk's
                    # t=0 write lands on fresh zeros) and stopped by the
                    # last-touched chunk.
                    for j8 in range(8):
                        c = half * 8 + j8
                        nc.tensor.matmul(
                            acc_ap(pv_banks, c, h),
                            ut[:, j8 * P : (j8 + 1) * P],
                            V4b[t][:, h * E : (h + 1) * E],
                            start=(t == 0 and c in (0, 7, 14)),
                            stop=(t == TT - 1 and c in (6, 13, 15)),
                        )

                first0_done = [False]  # bank B group start = first half-0 batch

                def drain(nmax):
                    n = 0
                    while n < nmax:
                        if pend[0] and pend[0][0][0] <= v_ready[0]:
                            emit_pv(*pend[0].pop(0))
                            first0_done[0] = True
                        elif (
                            pend[1]
                            and first0_done[0]
                            and pend[1][0][0] <= v_ready[0]
                            and not (pend[1][0][0] == TT - 1 and pend[0])
                        ):
                            emit_pv(*pend[1].pop(0))
                        else:
                            break
                        n += 1

                for k in range(32):
                    half, t = k // 16, k % 16
                    # PV batch first: it has no psum-slot dependency, so it
                    # fills the PE wait for exp(k-2) to free the scores slot.
                    kq = k - LAG
                    if kq >= 0:
                        pend[kq // 16].append((kq % 16, kq // 16, u_of.pop(kq)))
                    drain(
                        4 if k >= 28 else (3 if len(pend[0]) + len(pend[1]) > 4 else 2)
                    )
                    if prefill:
                        for fn in prefill.get(k, ()):
                            fn(v_ready)
                    # scores + exp
                    sc_t = ps_sc.tile([P, 1024], f32, tag="sc", name=f"sc{h}_{k}")
                    for j in range(2):
                        s0 = half * 1024 + j * 512
                        nc.tensor.matmul(
                            sc_t[:, j * 512 : (j + 1) * 512],
                            KT2[p][rp, t * P : (t + 1) * P],
                            QT2[p][rp, s0 : s0 + 512],
                            start=True,
                            stop=True,
                        )
                    ut = ub.tile([P, 1024], bf, tag="u", name=f"u{h}_{k}")
                    nc.scalar.activation(ut[:], sc_t[:], EXP, scale=scale)
                    u_of[k] = ut
                    # fillers
                    for fn in fillers.get(k, ()):
                        fn(v_ready)
                    if early_tail:
                        for es, fn in early_tail:
                            if es == k:
                                fn(v_ready)
                # flush remaining PV work
                for k in range(32 - LAG, 32):
                    pend[k // 16].append((k % 16, k // 16, u_of.pop(k)))
                drain(10**9)
                assert not pend[0] and not pend[1], (h, pend)

            # normalize + transpose, chunk-range granular (head 3 early-tails
            # only bank A's chunks 0..6 — reading an open-group bank is illegal)
            def norm_range(h, pv_banks, obfs, c0, c1):
                for c in range(c0, c1):
                    a = acc_ap(pv_banks, c, h)
                    rsb = smp.tile([P, 1], f32, tag="r", bufs=4, name=f"r{h}_{c}")
                    nc.vector.reciprocal(rsb[:], a[:, DK:E])
                    ob = smp.tile([P, DK], bf, tag="o", bufs=18, name=f"o{h}_{c}")
                    nc.vector.tensor_scalar_mul(ob[:], a[:, 0:DK], rsb[:, 0:1])
                    obfs[c] = ob

            def tr_range(h, obfs, c0, c1):
                p, r = h // 2, h % 2
                n = c1 - c0
                trt = ps_sp.tile([64, n * P], bf, tag="sp", name=f"tr{h}_{c0}")
                for i in range(n):
                    nc.tensor.transpose(
                        trt[:, i * P : (i + 1) * P], obfs.pop(c0 + i), id_sb[:]
                    )
                nc.vector.tensor_copy(
                    OT2[p][64 * r : 64 * r + 64, c0 * P : c1 * P], trt[0:64, :]
                )

            def make_norm_tr(h, pv_banks):
                obfs = {}

                def norm(g):
                    return lambda _vr: norm_range(h, pv_banks, obfs, 4 * g, 4 * g + 4)

                def tr(g):
                    return lambda _vr: tr_range(h, obfs, 4 * g, 4 * g + 4)

                return [norm(g) for g in range(4)] + [tr(g) for g in range(4)], obfs

            # Wo for one s-chunk m: contraction over both pairs, 2x 512-wide.
            # mode "dve": during head-3's second half (ACT is busy with exps):
            #   psum from the freed pvA bank + sp bank, drains on DVE only.
            # mode "tail": ps_sc rotation, drains split ACT/DVE in parallel.
            def wo_m(m, mode):
                msl = slice(m * P, (m + 1) * P)
                ot_b = outp.tile([P, D], bf, tag="outt", name=f"outt{m}")
                if mode == "dve":
                    wops = [
                        ps_pv.tile([P, 512], f32, tag="pvA", name=f"woA{m}"),
                        ps_sp.tile([P, 512], f32, tag="sp", name=f"woB{m}"),
                    ]
                elif mode == "tail2":
                    # second psum slot set for the tail so Wo(m) does not
                    # wait on drain(m-2) through the 2-slot sc rotation
                    wops = [
                        ps_pv.tile([P, 512], f32, tag="pvA", name=f"woA{m}"),
                        ps_pv.tile([P, 512], f32, tag="pvB", name=f"woB{m}"),
                    ]
                else:
                    w = ps_sc.tile([P, D], f32, tag="sc", name=f"wop{m}")
                    wops = [w[:, 0:512], w[:, 512:1024]]
                for dj in range(D // 512):
                    dsl = slice(dj * 512, (dj + 1) * 512)
                    for p in range(NPAIR):
                        nc.tensor.matmul(
                            wops[dj][:, 0:512] if mode == "dve" else wops[dj],
                            OT2[p][:, msl],
                            wo_sb[p][:, dsl],
                            start=(p == 0),
                            stop=(p == NPAIR - 1),
                        )
                for dj in range(D // 512):
                    dsl = slice(dj * 512, (dj + 1) * 512)
                    src = wops[dj] if mode == "tail" else wops[dj][:, 0:512]
                    if mode != "dve" and dj == 0:
                        nc.scalar.copy(ot_b[:, dsl], src)
                    else:
                        nc.vector.tensor_copy(ot_b[:, dsl], src)
                    if mode != "dve" and m >= 14:
                        # last tiles: per-half DMA right after each drain so
                        # the final transfer is small
                        dma(_stdy, out_d[msl, dsl], ot_b[:, dsl])
                if not (mode != "dve" and m >= 14):
                    dma(_stdy, out_d[msl, :], ot_b[:])

            # ---------------- filler schedules ------------------------------
            # A projection chain for cols c0:c1 is emitted as ~427ns pieces
            # (2 contraction chunks each): ACT is saturated during attention
            # and any PE burst between two scores delays every later exp
            # (a one-way ratchet), so fillers must stay below per-step slack.
            def mk_qk(p, which, c0, c1, pool, tag, eng="dve", npc=2):
                w = (wq_sb if which == "q" else wk_sb)[p]
                bsb = (bq_sb if which == "q" else bk_sb)[p]
                dst = (QT2 if which == "q" else KT2)[p]
                x = xq_sb if which == "q" else xk_sb
                st = {}
                pieces = []
                for k0 in range(0, KT, npc):
                    def piece(_vr, k0=k0):
                        if k0 == 0:
                            st["ps"] = pool.tile(
                                [P, 512], f32, tag=tag,
                                name=f"pj_{which}{p}_{c0}",
                            )
                        proj_chain_mm(w, x, c0, c1, st["ps"], k0, k0 + npc, bsb)
                        if k0 + npc == KT:
                            proj_copy(dst, c0, c1, st["ps"], eng)
                    pieces.append(piece)
                return pieces

            def v_group(g):
                """chain t=2g | chain t=2g+1 + epilogue (one step apart)."""
                st = {}

                def a(vr):
                    st["ps"] = ps_sp.tile([P, 512], f32, tag="sp", name=f"vps{g}")
                    v_chain(2 * g, st["ps"], 0, True, False)

                def b(vr):
                    v_chain(2 * g + 1, st["ps"], NV, False, True)
                    v_epilogue(2 * g, st["ps"], 0)
                    v_epilogue(2 * g + 1, st["ps"], NV)
                    vr[0] = 2 * g + 1

                return a, b

            # prefix: projections needed by head-0 step 0 (q s 0:1024, k t 0:256)
            for piece in mk_qk(0, "k", 0, 256, ps_sp, "sp", "act", npc=4):
                piece(None)
            for piece in mk_qk(0, "q", 0, 512, ps_sc, "sc", "act", npc=4):
                piece(None)
            for piece in mk_qk(0, "q", 512, 1024, ps_sc, "sc", "dve", npc=4):
                piece(None)

            def sched(f, pieces, steps):
                for fn, s in zip(pieces, steps):
                    f.setdefault(s, []).append(fn)

            fill0 = {}
            # dual psum slots (pvC + sp) let chains overlap; piece steps are
            # JIT vs the DMA stream; copies land >= 1 step before first use.
            prefill0 = {}
            sched(fill0, mk_qk(0, "k", 256, 512, ps_pv, "pvC", npc=4), (0, 1))
            sched(fill0, mk_qk(0, "k", 512, 1024, ps_sp, "sp"), (2, 2, 3, 3))
            sched(fill0, mk_qk(0, "k", 1024, 1536, ps_pv, "pvC"), (6, 6, 7, 7))
            sched(fill0, mk_qk(0, "k", 1536, 2048, ps_sp, "sp"), (10, 10, 11, 11))
            sched(fill0, mk_qk(0, "q", 1024, 1536, ps_pv, "pvC"), (13, 13, 14, 14))
            # q-s3 is DMA-gated until ~step 15; its last piece + copy run in
            # the pre-scores slot of step 16 (program order = dep order).
            qs3 = mk_qk(0, "q", 1536, 2048, ps_sp, "sp")
            sched(fill0, qs3[:3], (14, 15, 15))
            sched(prefill0, qs3[3:], (16,))
            for g, s in enumerate((18, 20, 22, 24, 26, 27, 28, 29)):
                a, b = v_group(g)
                fill0.setdefault(s, []).append(a)
                fill0.setdefault(min(s + 1, 31), []).append(b)

            def head_fill(prev_norm_tr, qk_items, stride=5):
                f = {}
                for i, fn in enumerate(prev_norm_tr):
                    f.setdefault(i // 2, []).append(fn)
                step = 4
                for p, which, sc, tag in qk_items:
                    pool = ps_pv if tag == "pvC" else ps_sp
                    pieces = mk_qk(p, which, sc * 512, (sc + 1) * 512, pool, tag)
                    sched(f, pieces, range(step, step + 4))
                    step += stride
                return f

            # ---------------- run the four heads ----------------------------
            def pv_alloc(h):
                return [
                    ps_pv.tile([P, 512], f32, tag="pvA", name=f"pvA_{h}"),
                    ps_pv.tile([P, 512], f32, tag="pvB", name=f"pvB_{h}"),
                    None,  # pvC: lazy, see acc_ap
                ]

            banks0 = pv_alloc(0)
            attn_head(0, fill0, banks0, prefill=prefill0)
            nt0, _ = make_norm_tr(0, banks0)
            banks1 = pv_alloc(1)
            attn_head(
                1,
                head_fill(
                    nt0,
                    [(1, "q", 0, "pvC"), (1, "k", 0, "sp"), (1, "q", 1, "pvC"),
                     (1, "k", 1, "sp"), (1, "k", 2, "sp")],
                ),
                banks1,
            )
            nt1, _ = make_norm_tr(1, banks1)
            banks2 = pv_alloc(2)
            attn_head(
                2,
                head_fill(
                    nt1,
                    [(1, "k", 3, "pvC"), (1, "q", 2, "sp"), (1, "q", 3, "pvC")],
                    stride=4,
                ),
                banks2,
            )
            nt2, _ = make_norm_tr(2, banks2)
            banks3 = pv_alloc(3)
            obfs3 = {}
            # head 3: bank A's chunks 0..6 finish (group closed) once half-0
            # PV flushes (~step 18): normalize/transpose them and run Wo
            # m=0..6 during half 1. Chunk 7 shares bank B with half-1 chunks
            # (group still open) so it waits for the tail.
            # early Wo as two 427ns half-pieces per m so it stays under the
            # per-step PE slack of head-3's ACT-paced second half
            def wo_m_dve(m):
                msl = slice(m * P, (m + 1) * P)
                st = {}

                def piece(dj):
                    def fn(_vr):
                        dsl = slice(dj * 512, (dj + 1) * 512)
                        if dj == 0:
                            st["ot"] = outp.tile(
                                [P, D], bf, tag="outt", name=f"outt{m}"
                            )
                            ps = ps_pv.tile([P, 512], f32, tag="pvA", name=f"woA{m}")
                        else:
                            ps = ps_sp.tile([P, 512], f32, tag="sp", name=f"woB{m}")
                        for p in range(NPAIR):
                            nc.tensor.matmul(
                                ps[:, 0:512],
                                OT2[p][:, msl],
                                wo_sb[p][:, dsl],
                                start=(p == 0),
                                stop=(p == NPAIR - 1),
                            )
                        nc.vector.tensor_copy(st["ot"][:, dsl], ps[:, 0:512])
                        if dj == 1:
                            dma(_stdy, out_d[msl, :], st["ot"][:])
                    return fn

                return piece(0), piece(1)

            early = [
                (19, lambda _vr: norm_range(3, banks3, obfs3, 0, 4)),
                (20, lambda _vr: norm_range(3, banks3, obfs3, 4, 7)),
                (20, lambda _vr: tr_range(3, obfs3, 0, 4)),
                (21, lambda _vr: tr_range(3, obfs3, 4, 7)),
            ]
            for m, (sa, sb) in enumerate(
                ((21, 22), (23, 24), (25, 26), (27, 28), (29, 29), (30, 30), (31, 31))
            ):
                pa, pb = wo_m_dve(m)
                early.append((sa, pa))
                early.append((sb, pb))
            attn_head(3, head_fill(nt2, []), banks3, early_tail=early)

            # ---------------- tail: chunks 7..15 of head 3, Wo m=7..15 ------
            # all normalizes/transposes first, then Wo streams through four
            # psum slots (sc x2 + pvA/pvB) without drain-WAR stalls
            norm_range(3, banks3, obfs3, 7, 8)
            tr_range(3, obfs3, 7, 8)
            wo_m(7, "tail")
            norm_range(3, banks3, obfs3, 8, 12)
            tr_range(3, obfs3, 8, 12)
            wo_m(8, "tail")
            norm_range(3, banks3, obfs3, 12, 16)
            wo_m(9, "tail2")
            tr_range(3, obfs3, 12, 16)
            for m in range(10, 16):
                wo_m(m, "tail" if m % 2 == 0 else "tail2")

    nc.finalize()
    return nc


def _prep_core_inputs(query, key, value, Wq, bq, Wk, bk, Wv, bv, Wo, b, g, NH, DK):
    """Host-side shard prep for core (b, g): transpose+cast, pack weights."""
    D = query.shape[2]
    h0 = g * NH
    sl = slice(h0, h0 + NH)
    Wq_g, Wk_g, Wv_g = Wq[sl], Wk[sl], Wv[sl]
    bq_g, bk_g, bv_g = bq[sl], bk[sl], bv[sl]
    NPAIR = NH // 2
    P = 128
    KT = D // P

    def pack_pair(W, bias):
        # [NPAIR, 128, D]: pair p cols = heads (2p, 2p+1) concat; k-major free
        w = np.concatenate(
            [
                np.concatenate([W[2 * p], W[2 * p + 1]], axis=1)[None]
                for p in range(NPAIR)
            ],
            axis=0,
        )  # [NPAIR, D, 128]
        w = w.reshape(NPAIR, KT, P, P).transpose(0, 2, 1, 3).reshape(NPAIR, P, D)
        bb = np.concatenate(
            [
                np.concatenate([bias[2 * p], bias[2 * p + 1]])[None, None]
                for p in range(NPAIR)
            ],
            axis=0,
        )  # [NPAIR, 1, 128]
        return w.astype(BF16), bb.astype(BF16)

    wq2, bq2 = pack_pair(Wq_g, bq_g)
    wk2, bk2 = pack_pair(Wk_g, bk_g)
    wv = np.concatenate([Wv_g[i] for i in range(NH)], axis=1)  # [D, NH*DK]
    NV = NH * DK
    wv4 = wv.reshape(KT, P, NV).transpose(1, 0, 2).reshape(P, KT * NV).astype(BF16)
    bv4 = np.concatenate([bv_g[i] for i in range(NH)])[None].astype(BF16)
    wo2 = (
        Wo[h0 * DK : (h0 + NH) * DK]
        .reshape(NPAIR, P, D)
        .astype(BF16)
    )
    return {
        "xqT": np.ascontiguousarray(query[b].T).astype(BF16),
        "xkT": np.ascontiguousarray(key[b].T).astype(BF16),
        "xvT": np.ascontiguousarray(value[b].T).astype(BF16),
        "wq2": wq2,
        "wk2": wk2,
        "wv4": wv4,
        "bq2": bq2,
        "bk2": bk2,
        "bv4": bv4,
        "wo2": wo2,
        "ident": np.eye(P, dtype=np.float32).astype(BF16),
    }


def kernel(query, key, value, Wq, bq, Wk, bk, Wv, bv, Wo, bo, _trace=False):
    from concourse.bass_utils import run_bass_kernel_spmd

    query = np.asarray(query, np.float32)
    key = np.asarray(key, np.float32)
    value = np.asarray(value, np.float32)
    B, S, D = query.shape
    H, _, DK = np.asarray(Wq).shape
    NCORE = 8
    GROUPS = NCORE // B
    NH = H // GROUPS

    with_bias = bool(
        np.any(np.asarray(bq)) or np.any(np.asarray(bk)) or np.any(np.asarray(bv))
    )
    ck = ("nc", with_bias)
    if ck not in _CACHE:
        _CACHE[ck] = _build_nc(S, D, DK, NH, with_bias=with_bias)
    nc = _CACHE[ck]

    in_maps = []
    for c in range(NCORE):
        b, g = c // GROUPS, c % GROUPS
        in_maps.append(
            _prep_core_inputs(
                np.asarray(query), np.asarray(key), np.asarray(value),
                np.asarray(Wq), np.asarray(bq), np.asarray(Wk), np.asarray(bk),
                np.asarray(Wv), np.asarray(bv), np.asarray(Wo), b, g, NH, DK,
            )
        )

    res = run_bass_kernel_spmd(nc, in_maps, list(range(NCORE)), trace=_trace)
    out = np.zeros((B, S, D), np.float32)
    for c in range(NCORE):
        out[c // GROUPS] += np.asarray(res.results[c]["out"], np.float32)
    out += np.asarray(bo, np.float32)[None, None, :]
    if _trace:
        _CACHE["last_results"] = res
    return out
